# revision 3
# baseline (speedup 1.0000x reference)
"""MoE top-1 routing kernel for Trainium2 (8 NeuronCores, expert-parallel).

Math (matches the reference):
    logits = x @ gate_w + gate_b            # [N, E]
    assign = argmax(logits, -1)             # top-1 expert per token
    out[t] = relu(x[t] @ w1[e] + b1[e]) @ w2[e] + b2[e]   where e = assign[t]

Strategy: the gate is a tiny (4096x1024x8) matmul computed on the host in
float64 (the smallest top1-top2 logit gap is ~2e-4, orders of magnitude above
fp32 rounding, so the argmax is unambiguous). Tokens are grouped by expert and
shipped to the core that holds that expert's weights (expert-parallel, one
expert per NeuronCore). Each core runs a dense 2-layer FFN over its token
group, padded to a uniform capacity so all 8 cores run one SPMD program.

Device kernel layout (per core, all fp32, matmuls in float32r):
    layer1: hT[m*128+p, c] = relu(sum_k w1[k,128 x m,128]^T @ xT[k,128 x c])
    layer2: yT[m2*128+p, c] = sum_k2 w2[k2,128 x m2,128]^T @ hT[k2,128 x c]
Both layers keep the contraction dim on SBUF partitions and tokens on the
free dim, so no on-device transposes are needed; the host pre-tiles the
weights so every DMA is fully contiguous per partition.
"""

import numpy as np

N_TOK, D, DFF, E = 4096, 1024, 4096, 8
P = 128
KD, KF = D // P, DFF // P  # 8, 32 contraction chunks

# test.py hooks: set TRACE=True (after installing the NTFF hook) to profile.
TRACE = False
LAST_RESULT = None

_PROGRAM_CACHE = {}


def _chunk_sizes(C):
    """Split C tokens into matmul moving-dim chunks (<=512 each, >=256 when
    possible so float32r runs at full rate)."""
    n = -(-C // 512)
    base, rem = divmod(C, n)
    return [base + (1 if i < rem else 0) for i in range(n)]


def _build_program(C):
    import concourse.mybir as mybir
    import concourse.tile as tile
    from concourse import bacc

    f32 = mybir.dt.float32
    f32r = mybir.dt.float32r
    AF = mybir.ActivationFunctionType

    chunks = _chunk_sizes(C)

    nc = bacc.Bacc("TRN2", target_bir_lowering=False, debug=False, num_devices=E)

    # float32r = fp32 bits, PE rounds to its reduced-precision fp32 mode; the
    # walrus verifier requires every producer feeding an f32r matmul input to
    # itself be typed f32r, so the whole matmul-input path is declared f32r.
    xt_d = nc.dram_tensor("xt", [P, KD * C], f32r, kind="ExternalInput").ap()
    w1_d = nc.dram_tensor("w1t", [KF, P, D], f32r, kind="ExternalInput").ap()
    b1_d = nc.dram_tensor("b1t", [P, KF], f32, kind="ExternalInput").ap()
    w2_d = nc.dram_tensor("w2t", [KD, P, DFF], f32r, kind="ExternalInput").ap()
    b2_d = nc.dram_tensor("b2t", [P, KD], f32, kind="ExternalInput").ap()
    yt_d = nc.dram_tensor("yt", [KD, P, C], f32, kind="ExternalOutput").ap()

    with tile.TileContext(nc) as tc:
        with (
            tc.tile_pool(name="xt_pool", bufs=1) as xt_pool,
            tc.tile_pool(name="ht_pool", bufs=1) as ht_pool,
            tc.tile_pool(name="w1_pool", bufs=3) as w1_pool,
            tc.tile_pool(name="w2_pool", bufs=2) as w2_pool,
            tc.tile_pool(name="y_pool", bufs=3) as y_pool,
            tc.tile_pool(name="bias_pool", bufs=1) as bias_pool,
            tc.tile_pool(name="psum", bufs=4, space="PSUM") as psum_pool,
        ):
            xt_sb = xt_pool.tile([P, KD * C], f32r)
            nc.sync.dma_start(xt_sb[:], xt_d[:])
            b1_sb = bias_pool.tile([P, KF], f32)
            nc.sync.dma_start(b1_sb[:], b1_d[:])
            b2_sb = bias_pool.tile([P, KD], f32)
            nc.sync.dma_start(b2_sb[:], b2_d[:])

            ht_sb = ht_pool.tile([P, KF * C], f32r)

            # layer 1: hT = relu(w1^T x^T + b1), m-th 128-row block of hT
            for m in range(KF):
                w1_sb = w1_pool.tile([P, D], f32r, tag="w1")
                nc.sync.dma_start(w1_sb[:], w1_d[m])
                t0 = 0
                for tn in chunks:
                    ps = psum_pool.tile([P, 512], f32, tag="ps")
                    for k in range(KD):
                        nc.tensor.matmul(
                            ps[:, :tn],
                            lhsT=w1_sb[:, k * P : (k + 1) * P],
                            rhs=xt_sb[:, k * C + t0 : k * C + t0 + tn],
                            start=(k == 0),
                            stop=(k == KD - 1),
                        )
                    nc.scalar.activation(
                        ht_sb[:, m * C + t0 : m * C + t0 + tn],
                        ps[:, :tn],
                        AF.Relu,
                        bias=b1_sb[:, m : m + 1],
                    )
                    t0 += tn

            # layer 2: yT = w2^T hT + b2, m2-th 128-row block of yT
            for m2 in range(KD):
                w2_sb = w2_pool.tile([P, DFF], f32r, tag="w2")
                nc.sync.dma_start(w2_sb[:], w2_d[m2])
                t0 = 0
                for tn in chunks:
                    ps2 = psum_pool.tile([P, 512], f32, tag="ps")
                    for k2 in range(KF):
                        nc.tensor.matmul(
                            ps2[:, :tn],
                            lhsT=w2_sb[:, k2 * P : (k2 + 1) * P],
                            rhs=ht_sb[:, k2 * C + t0 : k2 * C + t0 + tn],
                            start=(k2 == 0),
                            stop=(k2 == KF - 1),
                        )
                    yt_sb = y_pool.tile([P, 512], f32, tag="yt")
                    nc.scalar.activation(
                        yt_sb[:, :tn],
                        ps2[:, :tn],
                        AF.Identity,
                        bias=b2_sb[:, m2 : m2 + 1],
                    )
                    nc.sync.dma_start(yt_d[m2][:, t0 : t0 + tn], yt_sb[:, :tn])
                    t0 += tn

    nc.compile()
    return nc


def kernel(x, gate_w, gate_b, w1, b1, w2, b2):
    from concourse.bass_utils import run_bass_kernel_spmd

    global LAST_RESULT

    x = np.ascontiguousarray(np.asarray(x, dtype=np.float32))
    gate_w = np.asarray(gate_w, dtype=np.float32)
    gate_b = np.asarray(gate_b, dtype=np.float32)
    w1 = np.asarray(w1, dtype=np.float32)
    b1 = np.asarray(b1, dtype=np.float32)
    w2 = np.asarray(w2, dtype=np.float32)
    b2 = np.asarray(b2, dtype=np.float32)
    n_tok = x.shape[0]

    # host gate + top-1 routing (fp64: exact argmax, see module docstring)
    logits = x.astype(np.float64) @ gate_w.astype(np.float64) + gate_b.astype(
        np.float64
    )
    assign = np.argmax(logits, axis=-1)
    idx = [np.nonzero(assign == e)[0] for e in range(E)]
    cmax = max(1, max(len(i) for i in idx))
    C = max(256, -(-cmax // 64) * 64)

    if C not in _PROGRAM_CACHE:
        _PROGRAM_CACHE[C] = _build_program(C)
    nc = _PROGRAM_CACHE[C]

    in_maps = []
    for e in range(E):
        xe = np.zeros((C, D), np.float32)
        xe[: len(idx[e])] = x[idx[e]]
        # xt[p, k*C + c] = xe[c, k*128 + p]
        xt = np.ascontiguousarray(
            xe.T.reshape(KD, P, C).transpose(1, 0, 2).reshape(P, KD * C)
        )
        # w1t[m, p, k*128 + j] = w1[e][k*128 + p, m*128 + j]
        w1t = np.ascontiguousarray(
            w1[e].reshape(KD, P, KF, P).transpose(2, 1, 0, 3).reshape(KF, P, D)
        )
        # w2t[m2, p, k2*128 + j] = w2[e][k2*128 + p, m2*128 + j]
        w2t = np.ascontiguousarray(
            w2[e].reshape(KF, P, KD, P).transpose(2, 1, 0, 3).reshape(KD, P, DFF)
        )
        b1t = np.ascontiguousarray(b1[e].reshape(KF, P).T)
        b2t = np.ascontiguousarray(b2[e].reshape(KD, P).T)
        in_maps.append({"xt": xt, "w1t": w1t, "b1t": b1t, "w2t": w2t, "b2t": b2t})

    res = run_bass_kernel_spmd(nc, in_maps, core_ids=list(range(E)), trace=TRACE)
    LAST_RESULT = res

    out = np.zeros((n_tok, D), np.float32)
    for e in range(E):
        if len(idx[e]) == 0:
            continue
        yt = res.results[e]["yt"]  # [KD, P, C]
        ye = yt.transpose(2, 0, 1).reshape(C, D)
        out[idx[e]] = ye[: len(idx[e])]
    return out
